# revision 36
# baseline (speedup 1.0000x reference)
"""Patch-orthogonal-mix (unfold -> [L,D]@[D,D]^T -> fold) on 8 Trainium2 NeuronCores.

Strategy: pure data parallel over batch (2 images per core), weights replicated.
Per core, each image is processed in 8 horizontal strips of 32 pixel rows.

The unfold is realized by the input DMA layout: SBUF x-tiles hold partitions
p = ph_off*64 + c (ph_off = patch-row offset within a row-pair, c = channel),
so the patch-vector contraction dim d = (c, ph, pw) maps onto matmul
K-partitions across 8 accumulation steps (2 row-pairs x 4 pw columns), with
full-resolution rows loaded contiguously (1KB runs, no data duplication) and
cast f32->f16 inside the SWDGE DMA. A DVE copy then gathers the stride-4 pw
columns into contiguous blocks, because the PE streams a contiguous fp16
moving operand at 1 col/cycle but pays ~2x for strided access patterns.
Weights are host-packed so every lhsT is a plain [128,128] slice; fp16 weight
loads overlap the matmul stream completely. fp32 PSUM accumulation over the 8
K-chunks; the fold is realized by stride-4 interleaving PSUM->SBUF copies
(alternating scalar/vector engines) plus a mirrored output DMA pattern.
Steady state measures back-to-back matmuls at the 216 ns N=512 stream bound.
"""
import numpy as np

import concourse.bass as bass
import concourse.bacc as bacc
import concourse.mybir as mybir
from concourse.tile import TileContext
from concourse.bass_utils import run_bass_kernel_spmd

P = 4
C = 64
H = W = 256
B = 16
N_CORES = 8
B_LOC = B // N_CORES          # batches per core
STRIP = 32                    # pixel rows per strip
N_STRIPS = H // STRIP
HP_S = STRIP // P             # patch-rows per strip (8)
WP = W // P                   # patch-cols (64)
F32 = mybir.dt.float32
F32R = mybir.dt.float32r
F16 = mybir.dt.float16


def _build():
    nc = bacc.Bacc()
    x = nc.declare_dram_parameter("x", [B_LOC, C, H, W], F32, isOutput=False)
    w = nc.declare_dram_parameter("w", [128, 8192], F16, isOutput=False)
    y = nc.declare_dram_parameter("y", [B_LOC, C, H, W], F32, isOutput=True)

    with TileContext(nc) as tc:
        with (
            tc.tile_pool(name="wpool", bufs=1) as wpool,
            tc.tile_pool(name="xpool", bufs=2) as xpool,
            tc.tile_pool(name="gpool", bufs=6) as gpool,
            tc.tile_pool(name="spool", bufs=6) as spool,
            tc.tile_pool(name="psum", bufs=8, space="PSUM") as ppool,
        ):
            wt = wpool.tile([128, 8192], F16, tag="w")
            for j in range(8):
                nc.sync.dma_start(out=wt[:, j * 1024:(j + 1) * 1024],
                                  in_=w[:, j * 1024:(j + 1) * 1024])



            strips = [(b, 32 * k, 32) for b in range(B_LOC)
                      for k in range(N_STRIPS)]

            for b, r0, rows in strips:
                hp_s = rows // P
                n_l = hp_s * WP
                # rows of the strip grouped by h%4: [ph, c, hp, w]
                src4 = x[b, :, r0:r0 + rows, :].rearrange(
                    "c (hp ph) w -> ph c hp w", ph=P)
                xg = []
                for a in range(2):
                    t = xpool.tile([128, hp_s * 256], F16, tag="x")
                    for ph_off in range(2):
                        dst = t[ph_off * 64:(ph_off + 1) * 64, :].rearrange(
                            "p (hp w) -> p hp w", w=256)
                        # f32 -> f16 cast happens in the DMA (SWDGE only)
                        nc.gpsimd.dma_start(out=dst, in_=src4[2 * a + ph_off])
                    # gather pw-strided columns into contiguous blocks so the
                    # matmul rhs streams at 1 col/cycle
                    g = gpool.tile([128, hp_s * 256], F16, tag="xg")
                    nc.vector.tensor_copy(
                        out=g[:].rearrange("p (pw hp wp) -> p pw hp wp",
                                           hp=hp_s, wp=WP),
                        in_=t[:].rearrange("p (hp wp pw) -> p pw hp wp",
                                           wp=WP, pw=P),
                    )
                    xg.append(g)
                xr = [[g[:, pw * n_l:(pw + 1) * n_l] for pw in range(P)]
                      for g in xg]

                dsty4 = y[b, :, r0:r0 + rows, :].rearrange(
                    "c (hp ph) w -> ph c hp w", ph=P)
                for b2 in range(2):
                    st = spool.tile([128, hp_s * 256], F32, tag="st")
                    st_r = st[:].rearrange("p (hp wp pw) -> pw p (hp wp)",
                                           wp=WP, pw=P)
                    for pwp in range(P):
                        m_idx = b2 * P + pwp
                        ps = ppool.tile([128, n_l], F32)
                        step = 0
                        for a in range(2):
                            for pw in range(P):
                                f0 = ((a * 4 + pw) * 8 + m_idx) * 128
                                nc.tensor.matmul(
                                    ps[:],
                                    lhsT=wt[:, f0:f0 + 128],
                                    rhs=xr[a][pw],
                                    start=(step == 0),
                                    stop=(step == 7),
                                )
                                step += 1
                        if pwp % 2 == 0:
                            nc.scalar.copy(out=st_r[pwp], in_=ps[:])
                        else:
                            nc.vector.tensor_copy(out=st_r[pwp], in_=ps[:])
                    for php_off in range(2):
                        srcs = st[php_off * 64:(php_off + 1) * 64, :].rearrange(
                            "p (hp w) -> p hp w", w=256)
                        nc.sync.dma_start(out=dsty4[2 * b2 + php_off], in_=srcs)
    nc.compile()
    return nc


def _pack_w(W_mat):
    # lhsT layout: partitions p = ph_off*64 + c over the d-chunk
    # d = c*16 + (2a+ph_off)*4 + pw; free = (a, pw, b2, pwp, php_off, c') where
    # e = c'*16 + (2*b2+php_off)*4 + pwp.
    Wr = W_mat.reshape(64, 2, 2, 4, 64, 2, 2, 4)
    # axes: (c', b2, php_off, pwp, c, a, ph_off, pw)
    Wp = Wr.transpose(6, 4, 5, 7, 1, 3, 2, 0)
    # -> (ph_off, c, a, pw, b2, pwp, php_off, c')
    return np.ascontiguousarray(Wp.reshape(128, 8192).astype(np.float16))


_nc_cache = None


def _get_nc():
    global _nc_cache
    if _nc_cache is None:
        _nc_cache = _build()
    return _nc_cache


def _run(x, W_mat, trace=False, **kwargs):
    x = np.ascontiguousarray(np.asarray(x, dtype=np.float32))
    w_packed = _pack_w(np.ascontiguousarray(np.asarray(W_mat, dtype=np.float32)))
    nc = _get_nc()
    in_maps = [
        {"x": np.ascontiguousarray(x[i * B_LOC:(i + 1) * B_LOC]), "w": w_packed}
        for i in range(N_CORES)
    ]
    res = run_bass_kernel_spmd(nc, in_maps, list(range(N_CORES)), trace=trace,
                               **kwargs)
    y = np.concatenate([np.asarray(res.results[i]["y"]) for i in range(N_CORES)],
                       axis=0)
    return y, res


def kernel(**inputs):
    y, _ = _run(inputs["x"], inputs["W_mat"])
    return y


# revision 37
# speedup vs baseline: 1.0107x; 1.0107x over previous
"""Patch-orthogonal-mix (unfold -> [L,D]@[D,D]^T -> fold) on 8 Trainium2 NeuronCores.

Strategy: pure data parallel over batch (2 images per core), weights replicated.
Per core, each image is processed in 8 horizontal strips of 32 pixel rows.

The unfold is realized by the input DMA layout: SBUF x-tiles hold partitions
p = ph_off*64 + c (ph_off = patch-row offset within a row-pair, c = channel),
so the patch-vector contraction dim d = (c, ph, pw) maps onto matmul
K-partitions across 8 accumulation steps (2 row-pairs x 4 pw columns), with
full-resolution rows loaded contiguously (1KB runs, no data duplication) and
cast f32->f16 inside the SWDGE DMA. A DVE copy then gathers the stride-4 pw
columns into contiguous blocks, because the PE streams a contiguous fp16
moving operand at 1 col/cycle but pays ~2x for strided access patterns.
Weights are host-packed so every lhsT is a plain [128,128] slice; fp16 weight
loads overlap the matmul stream completely. fp32 PSUM accumulation over the 8
K-chunks; the fold is realized by stride-4 interleaving PSUM->SBUF copies
(alternating scalar/vector engines) plus a mirrored output DMA pattern.
Steady state measures back-to-back matmuls at the 216 ns N=512 stream bound.
"""
import numpy as np

import concourse.bass as bass
import concourse.bacc as bacc
import concourse.mybir as mybir
from concourse.tile import TileContext
from concourse.bass_utils import run_bass_kernel_spmd

P = 4
C = 64
H = W = 256
B = 16
N_CORES = 8
B_LOC = B // N_CORES          # batches per core
STRIP = 32                    # pixel rows per strip
N_STRIPS = H // STRIP
HP_S = STRIP // P             # patch-rows per strip (8)
WP = W // P                   # patch-cols (64)
F32 = mybir.dt.float32
F32R = mybir.dt.float32r
F16 = mybir.dt.float16


def _build():
    nc = bacc.Bacc()
    x = nc.declare_dram_parameter("x", [B_LOC, C, H, W], F32, isOutput=False)
    w = nc.declare_dram_parameter("w", [128, 8192], F16, isOutput=False)
    y = nc.declare_dram_parameter("y", [B_LOC, C, H, W], F32, isOutput=True)

    with TileContext(nc) as tc:
        with (
            tc.tile_pool(name="wpool", bufs=1) as wpool,
            tc.tile_pool(name="xpool", bufs=2) as xpool,
            tc.tile_pool(name="gpool", bufs=6) as gpool,
            tc.tile_pool(name="spool", bufs=6) as spool,
            tc.tile_pool(name="psum", bufs=8, space="PSUM") as ppool,
        ):
            wt = wpool.tile([128, 8192], F16, tag="w")
            for j in range(8):
                nc.sync.dma_start(out=wt[:, j * 1024:(j + 1) * 1024],
                                  in_=w[:, j * 1024:(j + 1) * 1024])



            strips = [(b, 32 * k, 32) for b in range(B_LOC)
                      for k in range(N_STRIPS)]

            for b, r0, rows in strips:
                hp_s = rows // P
                n_l = hp_s * WP
                # rows of the strip grouped by h%4: [ph, c, hp, w]
                src4 = x[b, :, r0:r0 + rows, :].rearrange(
                    "c (hp ph) w -> ph c hp w", ph=P)
                xg = []
                for a in range(2):
                    t = xpool.tile([128, hp_s * 256], F16, tag="x")
                    for ph_off in range(2):
                        dst = t[ph_off * 64:(ph_off + 1) * 64, :].rearrange(
                            "p (hp w) -> p hp w", w=256)
                        # f32 -> f16 cast happens in the DMA (SWDGE only)
                        nc.gpsimd.dma_start(out=dst, in_=src4[2 * a + ph_off])
                    # gather pw-strided columns into contiguous blocks so the
                    # matmul rhs streams at 1 col/cycle
                    g = gpool.tile([128, hp_s * 256], F16, tag="xg")
                    for half in range(2):
                        nc.vector.tensor_copy(
                            out=g[:, half * 2 * n_l:(half + 1) * 2 * n_l].rearrange(
                                "p (pw hp wp) -> p pw hp wp", hp=hp_s, wp=WP),
                            in_=t[:].rearrange("p (hp wp pw) -> p pw hp wp",
                                               wp=WP, pw=P)[:, half * 2:(half + 1) * 2],
                        )
                    xg.append(g)
                xr = [[g[:, pw * n_l:(pw + 1) * n_l] for pw in range(P)]
                      for g in xg]

                dsty4 = y[b, :, r0:r0 + rows, :].rearrange(
                    "c (hp ph) w -> ph c hp w", ph=P)
                for b2 in range(2):
                    st = spool.tile([128, hp_s * 256], F32, tag="st")
                    st_r = st[:].rearrange("p (hp wp pw) -> pw p (hp wp)",
                                           wp=WP, pw=P)
                    for pwp in range(P):
                        m_idx = b2 * P + pwp
                        ps = ppool.tile([128, n_l], F32)
                        step = 0
                        for a in range(2):
                            for pw in range(P):
                                f0 = ((a * 4 + pw) * 8 + m_idx) * 128
                                nc.tensor.matmul(
                                    ps[:],
                                    lhsT=wt[:, f0:f0 + 128],
                                    rhs=xr[a][pw],
                                    start=(step == 0),
                                    stop=(step == 7),
                                )
                                step += 1
                        if pwp % 2 == 0:
                            nc.scalar.copy(out=st_r[pwp], in_=ps[:])
                        else:
                            nc.vector.tensor_copy(out=st_r[pwp], in_=ps[:])
                    for php_off in range(2):
                        srcs = st[php_off * 64:(php_off + 1) * 64, :].rearrange(
                            "p (hp w) -> p hp w", w=256)
                        nc.sync.dma_start(out=dsty4[2 * b2 + php_off], in_=srcs)
    nc.compile()
    return nc


def _pack_w(W_mat):
    # lhsT layout: partitions p = ph_off*64 + c over the d-chunk
    # d = c*16 + (2a+ph_off)*4 + pw; free = (a, pw, b2, pwp, php_off, c') where
    # e = c'*16 + (2*b2+php_off)*4 + pwp.
    Wr = W_mat.reshape(64, 2, 2, 4, 64, 2, 2, 4)
    # axes: (c', b2, php_off, pwp, c, a, ph_off, pw)
    Wp = Wr.transpose(6, 4, 5, 7, 1, 3, 2, 0)
    # -> (ph_off, c, a, pw, b2, pwp, php_off, c')
    return np.ascontiguousarray(Wp.reshape(128, 8192).astype(np.float16))


_nc_cache = None


def _get_nc():
    global _nc_cache
    if _nc_cache is None:
        _nc_cache = _build()
    return _nc_cache


def _run(x, W_mat, trace=False, **kwargs):
    x = np.ascontiguousarray(np.asarray(x, dtype=np.float32))
    w_packed = _pack_w(np.ascontiguousarray(np.asarray(W_mat, dtype=np.float32)))
    nc = _get_nc()
    in_maps = [
        {"x": np.ascontiguousarray(x[i * B_LOC:(i + 1) * B_LOC]), "w": w_packed}
        for i in range(N_CORES)
    ]
    res = run_bass_kernel_spmd(nc, in_maps, list(range(N_CORES)), trace=trace,
                               **kwargs)
    y = np.concatenate([np.asarray(res.results[i]["y"]) for i in range(N_CORES)],
                       axis=0)
    return y, res


def kernel(**inputs):
    y, _ = _run(inputs["x"], inputs["W_mat"])
    return y
